# revision 15
# baseline (speedup 1.0000x reference)
"""Trainium2 Bass kernel for an AttentionBlock (GroupNorm -> q/k/v 1x1 conv ->
full S x S attention -> proj 1x1 conv -> residual).

Problem shapes: x [4, 512, 64, 64] fp32, S = 4096 tokens, C = 512 channels,
GroupNorm with 32 groups of 16 channels.

Sharding: 8 cores = 4 batches x 2 query-halves. Core c handles batch c//2 and
query rows [half*2048, (half+1)*2048). Each core of a batch-pair redundantly
computes k/v for its batch (cheap vs attention) so no collectives are needed.

Math/precision design (validated against the reference in numpy, rel-maxerr
~9e-3 vs the 2e-2 gate):
  * GroupNorm folded into the q/k/v weights: h = scale_c * x + shift_c, so
    q = (wq*scale) @ x + (bq + wq @ shift), etc. Stats are computed from the
    fp8 copy of x (adds ~0.1% to variance - negligible).
  * k's bias adds a per-query softmax constant -> cancels, never computed.
  * v's bias times sum_j(p_j) = 1 -> folded into the proj bias.
  * All big matmuls run in fp8e4m3 with DoubleRow perf mode (2 fp8 values per
    PE cell, contraction 256/instruction at 0.5 cycles/row).
  * Softmax: ex' = exp(s/sqrt(C) - KOFF) written by the ACT engine DIRECTLY to
    fp8 (KOFF chosen so max ex' ~ 30 << 240 = e4m3 max, and every row-max
    stays in normal range). The denominator is the PE-summed (ones-matmul,
    DoubleRow) total of the QUANTIZED ex' -> dividing after attn@v removes
    first-order quantization bias. Normalization: hs = ph * (1/den) on DVE.
  * proj runs f32r with f32r hs; residual + proj bias added by one DVE
    scalar_tensor_tensor per output tile.

Layouts (partition dim first; "DR pair" = 2 half-tiles packed for DoubleRow):
  x8  [m=2][cpair=128, u=2, s=4096]   channel c = m*256 + u*128 + p
  k8  [mo=2][128, 2, j=4096]          out-channel pairs, scores lhsT
  q8  [mo=2][128, 2, i=2048]          scores rhs
  v8  [j=128, t=16, u=2, c=512]       j = (2t+u)*128 + p, attn@v lhsT
  ex8 [j=128, t=16, u=2, i=512]       attn@v rhs (per i-block of 512)
"""

import os

import numpy as np
import ml_dtypes

import concourse.bacc as bacc
import concourse.tile as tile
from concourse import mybir
from concourse.bass_utils import run_bass_kernel_spmd

F32 = mybir.dt.float32
F32R = mybir.dt.float32r
FP8 = mybir.dt.float8e4
AF = mybir.ActivationFunctionType
OP = mybir.AluOpType
AX = mybir.AxisListType
DR = mybir.MatmulPerfMode.DoubleRow

C = 512
S = 4096
B = 4
NCORES = 8
CT = 4          # channel tiles of 128
CP = 2          # channel pair-tiles of 256 (DoubleRow)
SBLK = 8        # s-blocks of 512
QBLK = 4        # q-blocks of 512 (half = 2048 columns)
IB = 4          # i-blocks of 512 for attention
IBW = 512
JT = 32         # j-tiles of 128
TP = 16         # j pair-tiles of 256
HALF = S // 2
EPS = 1e-5
GELEMS = 16 * S                      # elements per group (16 ch x 4096)
SCL = 1.0 / np.sqrt(np.float32(C))   # softmax scale
KOFF = 4.0                           # exp offset: ex' = exp(s*SCL - KOFF)
# Timing-bisection variants (correctness only guaranteed for "full"):
#   full | nop5 (P1-P4 only) | p5a (no attnV/proj) | p5b (no scores/exp)
VARIANT = os.environ.get("KVARIANT", "full")


def build_nc(reps=1):
    nc = bacc.Bacc("TRN2", target_bir_lowering=False, debug=False,
                   num_devices=NCORES)

    x8_d = nc.dram_tensor("x8", [CP, 128, 2, S], FP8, kind="ExternalInput").ap()
    xh_d = nc.dram_tensor("xh", [CT, 128, HALF], F32, kind="ExternalInput").ap()
    wqt_d = nc.dram_tensor("wqt", [CT, 128, C], F32, kind="ExternalInput").ap()
    wkt_d = nc.dram_tensor("wkt", [CT, 128, C], F32, kind="ExternalInput").ap()
    wvt_d = nc.dram_tensor("wvt", [CT, 128, C], F32, kind="ExternalInput").ap()
    wpt_d = nc.dram_tensor("wpt", [CT, 128, C], F32R, kind="ExternalInput").ap()
    bq_d = nc.dram_tensor("bq", [CT, 128, 1], F32, kind="ExternalInput").ap()
    bv_d = nc.dram_tensor("bv", [CT, 128, 1], F32, kind="ExternalInput").ap()
    bp_d = nc.dram_tensor("bp", [CT, 128, 1], F32, kind="ExternalInput").ap()
    gnw_d = nc.dram_tensor("gnw", [CT, 128, 1], F32, kind="ExternalInput").ap()
    gnb_d = nc.dram_tensor("gnb", [CT, 128, 1], F32, kind="ExternalInput").ap()
    g16_d = nc.dram_tensor("g16", [128, 8], F32, kind="ExternalInput").ap()
    b8_d = nc.dram_tensor("b8", [8, 128], F32, kind="ExternalInput").ap()
    on8_d = nc.dram_tensor("on8", [128, 2, 128], FP8, kind="ExternalInput").ap()
    out_d = nc.dram_tensor("out", [CT, 128, HALF], F32, kind="ExternalOutput").ap()

    with tile.TileContext(nc) as tc:
        with tc.tile_pool(name="const", bufs=1) as cpool, \
             tc.tile_pool(name="resident", bufs=1) as rpool:
            g16_t = cpool.tile([128, 8], F32, name="g16t")
            b8_t = cpool.tile([8, 128], F32, name="b8t")
            on8_t = cpool.tile([128, 2, 128], FP8, name="on8t")
            eps_t = cpool.tile([8, 1], F32, name="epst")
            koff_t = cpool.tile([128, 1], F32, name="kofft")
            nc.sync.dma_start(g16_t[:], g16_d[:])
            nc.sync.dma_start(b8_t[:], b8_d[:])
            nc.sync.dma_start(on8_t[:], on8_d[:])
            nc.vector.memset(eps_t[:], EPS)
            nc.vector.memset(koff_t[:], -KOFF)
            gnw_t, gnb_t = [], []
            for ci in range(CT):
                gw = cpool.tile([128, 1], F32, name=f"gnw{ci}")
                gb = cpool.tile([128, 1], F32, name=f"gnb{ci}")
                nc.sync.dma_start(gw[:], gnw_d[ci])
                nc.sync.dma_start(gb[:], gnb_d[ci])
                gnw_t.append(gw)
                gnb_t.append(gb)

            for rep in range(reps):
                emit_rep(nc, tc, rpool, rep,
                         x8_d, xh_d, wqt_d, wkt_d, wvt_d, wpt_d,
                         bq_d, bv_d, bp_d,
                         g16_t, b8_t, on8_t, eps_t, koff_t, gnw_t, gnb_t,
                         out_d)
    nc.compile()
    return nc


def emit_rep(nc, tc, rpool, rep, x8_d, xh_d, wqt_d, wkt_d, wvt_d, wpt_d,
             bq_d, bv_d, bp_d, g16_t, b8_t, on8_t, eps_t, koff_t,
             gnw_t, gnb_t, out_d):
    # ---- resident tensors (slots shared across reps via fixed tags) ----
    k8 = [rpool.tile([128, 2, S], FP8, name=f"k8{m}_{rep}", tag=f"k8{m}")
          for m in range(CP)]
    q8 = [rpool.tile([128, 2, HALF], FP8, name=f"q8{m}_{rep}", tag=f"q8{m}")
          for m in range(CP)]
    v8 = rpool.tile([128, TP, 2, C], FP8, name=f"v8_{rep}", tag="v8")
    wpt_s = [rpool.tile([128, C], F32R, name=f"wp{ci}_{rep}", tag=f"wp{ci}")
             for ci in range(CT)]
    for ci in range(CT):
        nc.sync.dma_start(wpt_s[ci][:], wpt_d[ci])

    with tc.tile_pool(name=f"x8_{rep}", bufs=1) as x8pool, \
         tc.tile_pool(name=f"stat_{rep}", bufs=1) as spool, \
         tc.tile_pool(name=f"pstat_{rep}", bufs=1, space="PSUM") as pstats:

        x8_s = [x8pool.tile([128, 2, S], FP8, name=f"x8s{m}_{rep}", tag=f"x8m{m}")
                for m in range(CP)]
        for m in range(CP):
            nc.sync.dma_start(x8_s[m][:], x8_d[m])

        # ================= P1: per-channel sum / sumsq from fp8 x ============
        sq2 = spool.tile([128, CT, 2], F32, name=f"sq2_{rep}", tag="sq2")
        sqscr = spool.tile([128, S], FP8, name=f"sqscr_{rep}", tag="sqscr")
        for ci in range(CT):
            m, u = ci // 2, ci % 2
            xv = x8_s[m][:, u, :]
            nc.vector.reduce_sum(out=sq2[:, ci, 0:1], in_=xv, axis=AX.X)
            nc.scalar.activation(out=sqscr[:], in_=xv, func=AF.Square,
                                 accum_out=sq2[:, ci, 1:2])

        # ================= P2: group stats -> per-channel scale/shift =========
        gpsum = pstats.tile([8, 8], F32, name=f"gps_{rep}", tag="g")
        for ci in range(CT):
            nc.tensor.matmul(gpsum[:, 2 * ci:2 * ci + 2], g16_t[:], sq2[:, ci, :],
                             start=True, stop=True)
        gp3 = gpsum[:].rearrange("p (c t) -> p c t", t=2)
        packbuf = spool.tile([8, CT, 2], F32, name=f"pack_{rep}", tag="pack")
        ex2 = spool.tile([8, CT], F32, name=f"ex2_{rep}", tag="ex2")
        gm2 = spool.tile([8, CT], F32, name=f"gm2_{rep}", tag="gm2")
        gvar = spool.tile([8, CT], F32, name=f"gvar_{rep}", tag="gvar")
        nc.scalar.mul(out=packbuf[:, :, 1], in_=gp3[:, :, 0], mul=1.0 / GELEMS)
        nc.scalar.mul(out=ex2[:], in_=gp3[:, :, 1], mul=1.0 / GELEMS)
        nc.vector.tensor_mul(gm2[:], packbuf[:, :, 1], packbuf[:, :, 1])
        nc.vector.tensor_sub(gvar[:], ex2[:], gm2[:])
        nc.scalar.activation(out=gvar[:], in_=gvar[:], func=AF.Sqrt,
                             bias=eps_t[:], scale=1.0)
        nc.vector.reciprocal(out=packbuf[:, :, 0], in_=gvar[:])
        scale_t, shift_t = [], []
        for ci in range(CT):
            bca = pstats.tile([128, 2], F32, name=f"bca{ci}_{rep}", tag="bca")
            nc.tensor.matmul(bca[:], b8_t[:], packbuf[:, ci, :], start=True, stop=True)
            sc = spool.tile([128, 1], F32, name=f"scale{ci}_{rep}", tag=f"scale{ci}")
            sh = spool.tile([128, 1], F32, name=f"shift{ci}_{rep}", tag=f"shift{ci}")
            tm = spool.tile([128, 1], F32, name=f"tmpm{ci}_{rep}", tag="tmpm")
            nc.vector.tensor_mul(sc[:], gnw_t[ci][:], bca[:, 0:1])
            nc.vector.tensor_mul(tm[:], bca[:, 1:2], sc[:])
            nc.vector.tensor_sub(sh[:], gnb_t[ci][:], tm[:])
            scale_t.append(sc)
            shift_t.append(sh)

        # ================= P3: fold GN into fp8 weights + bias folds ==========
        with tc.tile_pool(name=f"w8_{rep}", bufs=1) as w8pool:
            wq8 = [w8pool.tile([128, 2, C], FP8, name=f"wq8{m}_{rep}", tag=f"wq8{m}")
                   for m in range(CP)]
            wk8 = [w8pool.tile([128, 2, C], FP8, name=f"wk8{m}_{rep}", tag=f"wk8{m}")
                   for m in range(CP)]
            wv8 = [w8pool.tile([128, 2, C], FP8, name=f"wv8{m}_{rep}", tag=f"wv8{m}")
                   for m in range(CP)]
            with tc.tile_pool(name=f"wfold_{rep}", bufs=1) as wfold:
                wq_s, wk_s, wv_s = [], [], []
                for nm, src, lst in (("wq", wqt_d, wq_s), ("wk", wkt_d, wk_s),
                                     ("wv", wvt_d, wv_s)):
                    for ci in range(CT):
                        w = wfold.tile([128, C], F32, name=f"{nm}{ci}_{rep}",
                                       tag=f"{nm}{ci}")
                        nc.sync.dma_start(w[:], src[ci])
                        lst.append(w)
                # bias folds with RAW weights: b' = b + w^T @ shift
                bq_sb, bv_sb = [], []
                for w_s, b_dram, lst, nm in ((wq_s, bq_d, bq_sb, "bq"),
                                             (wv_s, bv_d, bv_sb, "bv")):
                    for co in range(CT):
                        pb = pstats.tile([128, 1], F32, name=f"pb{nm}{co}_{rep}",
                                         tag="pb")
                        for ci in range(CT):
                            nc.tensor.matmul(
                                pb[:], w_s[ci][:, co * 128:(co + 1) * 128],
                                shift_t[ci][:], start=(ci == 0), stop=(ci == CT - 1))
                        braw = spool.tile([128, 1], F32, name=f"{nm}r{co}_{rep}",
                                          tag="braw")
                        nc.sync.dma_start(braw[:], b_dram[co])
                        bt = spool.tile([128, 1], F32, name=f"{nm}f{co}_{rep}",
                                        tag=f"{nm}f{co}")
                        nc.vector.tensor_add(bt[:], pb[:], braw[:])
                        lst.append(bt)
                # bp' = bp + wp^T @ bv'
                bp_sb = []
                for co in range(CT):
                    pb = pstats.tile([128, 1], F32, name=f"pbbp{co}_{rep}", tag="pb")
                    for ci in range(CT):
                        nc.tensor.matmul(
                            pb[:], wpt_s[ci][:].bitcast(F32)[:, co * 128:(co + 1) * 128],
                            bv_sb[ci][:], start=(ci == 0), stop=(ci == CT - 1))
                    braw = spool.tile([128, 1], F32, name=f"bpr{co}_{rep}", tag="braw")
                    nc.sync.dma_start(braw[:], bp_d[co])
                    bt = rpool.tile([128, 1], F32, name=f"bpf{co}_{rep}",
                                    tag=f"bpf{co}")
                    nc.vector.tensor_add(bt[:], pb[:], braw[:])
                    bp_sb.append(bt)
                # folded fp8 weights: w8[m][:, u, :] = w_s[2m+u] * scale[2m+u]
                # (on ACT - Copy with per-partition scale - to keep DVE free)
                for ws, w8t in ((wq_s, wq8), (wk_s, wk8), (wv_s, wv8)):
                    for ci in range(CT):
                        nc.scalar.activation(
                            out=w8t[ci // 2][:, ci % 2, :], in_=ws[ci][:],
                            func=AF.Copy, scale=scale_t[ci][:])

            # ================= P4: q / k / vT projections (DoubleRow) =========
            # k and q first (P5 stage-A needs them); v last with DVE stores so
            # the v work drains while P5's first exp-bound phase runs.
            with tc.tile_pool(name=f"pd_{rep}", bufs=5, space="PSUM") as pd:
                for sb in range(SBLK):
                    ssl = slice(sb * 512, (sb + 1) * 512)
                    for co in range(CT):
                        pk = pd.tile([128, 512], F32, name=f"pk{sb}{co}_{rep}",
                                     tag="pd")
                        for m in range(CP):
                            for u in range(2):
                                nc.tensor.matmul(
                                    pk[:], wk8[m][:, u, co * 128:(co + 1) * 128],
                                    x8_s[m][:, u, ssl], start=(m == 0 and u == 0),
                                    stop=(m == CP - 1 and u == 1))
                        if co % 2 == 0:
                            nc.scalar.activation(out=k8[co // 2][:, co % 2, ssl],
                                                 in_=pk[:], func=AF.Copy)
                        else:
                            nc.vector.tensor_copy(k8[co // 2][:, co % 2, ssl],
                                                  pk[:])
                    if sb < QBLK:
                        for co in range(CT):
                            pq = pd.tile([128, 512], F32, name=f"pq{sb}{co}_{rep}",
                                         tag="pd")
                            for m in range(CP):
                                for u in range(2):
                                    nc.tensor.matmul(
                                        pq[:],
                                        wq8[m][:, u, co * 128:(co + 1) * 128],
                                        x8_s[m][:, u, ssl],
                                        start=(m == 0 and u == 0),
                                        stop=(m == CP - 1 and u == 1))
                            nc.vector.tensor_scalar(
                                out=q8[co // 2][:, co % 2, ssl], in0=pq[:],
                                scalar1=bq_sb[co][:], scalar2=None, op0=OP.add)
                for jt in range(JT):
                    pv = pd.tile([128, 512], F32, name=f"pv{jt}_{rep}", tag="pd")
                    for m in range(CP):
                        for u in range(2):
                            nc.tensor.matmul(
                                pv[:], x8_s[m][:, u, jt * 128:(jt + 1) * 128],
                                wv8[m][:, u, :], start=(m == 0 and u == 0),
                                stop=(m == CP - 1 and u == 1))
                    nc.vector.tensor_copy(v8[:, jt // 2, jt % 2, :], pv[:])

    # ================= P5: attention + proj + residual (pipelined) ===========
    with tc.tile_pool(name=f"ex8_{rep}", bufs=2) as ex8pool, \
         tc.tile_pool(name=f"hn_{rep}", bufs=2) as hnpool, \
         tc.tile_pool(name=f"eo_{rep}", bufs=4) as eopool, \
         tc.tile_pool(name=f"psc_{rep}", bufs=3, space="PSUM") as psc, \
         tc.tile_pool(name=f"pph_{rep}", bufs=4, space="PSUM") as pph, \
         tc.tile_pool(name=f"psm_{rep}", bufs=1, space="PSUM") as psm:

        ex8_t, rbc_t = [None] * IB, [None] * IB

        def emit_slot(ia, ib):
            """One pipeline slot: stage-A of i-block `ia` (scores -> exp ->
            den) interleaved at js/t granularity with stage-B of i-block `ib`
            (attnV -> normalize -> proj -> residual). The interleave keeps the
            PE busy on attnV matmuls whenever the scores stream stalls on the
            (slower) ACT exp consumer, and vice versa."""
            if ia is not None:
                isl_a = slice(ia * IBW, (ia + 1) * IBW)
                ex8a = ex8pool.tile([128, TP, 2, IBW], FP8, name=f"ex{ia}_{rep}",
                                    tag="ex8")
                pdn = psm.tile([128, IBW], F32, name=f"pdn{ia}_{rep}", tag="sm")
            if ib is not None:
                isl_b = slice(ib * IBW, (ib + 1) * IBW)
                ex8b, rbcb = ex8_t[ib], rbc_t[ib]
                xrs = []
                for co in range(CT):
                    xr = eopool.tile([128, IBW], F32, name=f"xr{ib}{co}_{rep}",
                                     tag="xr")
                    nc.sync.dma_start(xr[:], xh_d[co, :, isl_b])
                    xrs.append(xr)
                ph = [pph.tile([128, IBW], F32, name=f"ph{ib}{ci}_{rep}", tag="ph")
                      for ci in range(CT)]
                hs = hnpool.tile([128, CT, IBW], F32R, name=f"hs{ib}_{rep}",
                                 tag="hs")
            for s in range(JT):
                if ia is not None:
                    ps_ = psc.tile([128, IBW], F32, name=f"ps{ia}{s}_{rep}",
                                   tag="ps")
                    for m in range(CP):
                        for u in range(2):
                            nc.tensor.matmul(
                                ps_[:], k8[m][:, u, s * 128:(s + 1) * 128],
                                q8[m][:, u, isl_a], start=(m == 0 and u == 0),
                                stop=(m == CP - 1 and u == 1))
                    nc.scalar.activation(out=ex8a[:, s // 2, s % 2, :], in_=ps_[:],
                                         func=AF.Exp, scale=float(SCL),
                                         bias=koff_t[:])
                    t, u = s // 2, s % 2
                    nc.tensor.matmul(pdn[:], on8_t[:, 0, :], ex8a[:, t, u, :],
                                     start=(s == 0), stop=(s == JT - 1),
                                     skip_group_check=True)
                if ib is not None:
                    for g in (2 * s, 2 * s + 1):
                        ci, t = g // TP, g % TP
                        for u in range(2):
                            nc.tensor.matmul(
                                ph[ci][:], v8[:, t, u, ci * 128:(ci + 1) * 128],
                                ex8b[:, t, u, :], start=(t == 0 and u == 0),
                                stop=(t == TP - 1 and u == 1),
                                skip_group_check=True)
                        if t == TP - 1:
                            nc.vector.tensor_mul(hs[:, ci, :], ph[ci][:], rbcb[:])
            if ia is not None:
                rbc = hnpool.tile([128, IBW], F32, name=f"rbc{ia}_{rep}",
                                  tag="rbc")
                nc.vector.reciprocal(out=rbc[:], in_=pdn[:])
                ex8_t[ia], rbc_t[ia] = ex8a, rbc
            if ib is not None:
                for co in range(CT):
                    pp = psc.tile([128, IBW], F32, name=f"pp{ib}{co}_{rep}",
                                  tag="ps")
                    for ci in range(CT):
                        nc.tensor.matmul(pp[:],
                                         wpt_s[ci][:, co * 128:(co + 1) * 128],
                                         hs[:, ci, :], start=(ci == 0),
                                         stop=(ci == CT - 1))
                    ot = eopool.tile([128, IBW], F32, name=f"ot{ib}{co}_{rep}",
                                     tag="ot")
                    nc.vector.scalar_tensor_tensor(out=ot[:], in0=pp[:],
                                                   scalar=bp_sb[co][:],
                                                   in1=xrs[co][:],
                                                   op0=OP.add, op1=OP.add)
                    nc.sync.dma_start(out_d[co, :, isl_b], ot[:])

        if VARIANT == "nop5":
            return
        if VARIANT == "p5a":
            for ib in range(IB):
                emit_slot(ib, None)
        elif VARIANT == "p5b":
            ex8c = rpool.tile([128, TP, 2, IBW], FP8, name=f"ex8c_{rep}",
                              tag="ex8c")
            rbcc = rpool.tile([128, IBW], F32, name=f"rbcc_{rep}", tag="rbcc")
            if rep == 0:
                nc.vector.memset(ex8c[:], 0.01)
                nc.vector.memset(rbcc[:], 1.0)
            for ib in range(IB):
                ex8_t[ib], rbc_t[ib] = ex8c, rbcc
            for ib in range(IB):
                emit_slot(None, ib)
        else:
            for ib in range(IB + 1):
                emit_slot(ib if ib < IB else None, ib - 1 if ib >= 1 else None)


# ---------------------------------------------------------------------------
# Host side
# ---------------------------------------------------------------------------
_NC_CACHE = {}


def _get_nc(reps=1):
    if reps not in _NC_CACHE:
        _NC_CACHE[reps] = build_nc(reps)
    return _NC_CACHE[reps]


def make_in_maps(x, gn_w, gn_b, wq, bq, wk, bk, wv, bv, wp, bp):
    xf = np.ascontiguousarray(np.asarray(x, dtype=np.float32)).reshape(B, C, S)
    g16 = np.zeros((128, 8), np.float32)
    g16[np.arange(128), np.arange(128) // 16] = 1.0
    b8 = np.ascontiguousarray(g16.T)
    shared = {
        "wqt": np.ascontiguousarray(np.asarray(wq, np.float32).T).reshape(CT, 128, C),
        "wkt": np.ascontiguousarray(np.asarray(wk, np.float32).T).reshape(CT, 128, C),
        "wvt": np.ascontiguousarray(np.asarray(wv, np.float32).T).reshape(CT, 128, C),
        "wpt": np.ascontiguousarray(np.asarray(wp, np.float32).T).reshape(CT, 128, C),
        "bq": np.asarray(bq, np.float32).reshape(CT, 128, 1),
        "bv": np.asarray(bv, np.float32).reshape(CT, 128, 1),
        "bp": np.asarray(bp, np.float32).reshape(CT, 128, 1),
        "gnw": np.asarray(gn_w, np.float32).reshape(CT, 128, 1),
        "gnb": np.asarray(gn_b, np.float32).reshape(CT, 128, 1),
        "g16": g16,
        "b8": b8,
        "on8": np.ones((128, 2, 128), ml_dtypes.float8_e4m3),
    }
    in_maps = []
    for core in range(NCORES):
        b, half = core // 2, core % 2
        xb = xf[b]
        if half == 0:
            xp = xb
        else:
            xp = np.concatenate([xb[:, HALF:], xb[:, :HALF]], axis=1)
        xp = np.ascontiguousarray(xp)
        # x8[m][p, u, s] = xp[m*256 + u*128 + p, s]
        x8 = np.ascontiguousarray(
            xp.reshape(CP, 2, 128, S).transpose(0, 2, 1, 3)
        ).astype(ml_dtypes.float8_e4m3)
        xh = np.ascontiguousarray(xp[:, :HALF]).reshape(CT, 128, HALF)
        in_maps.append(dict(shared, x8=x8, xh=xh))
    return in_maps


def assemble_out(results, H=64, W=64):
    out = np.empty((B, C, S), np.float32)
    for core in range(NCORES):
        b, half = core // 2, core % 2
        out[b][:, half * HALF:(half + 1) * HALF] = \
            results[core]["out"].reshape(C, HALF)
    return out.reshape(B, C, H, W)


def kernel(x, gn_w, gn_b, wq, bq, wk, bk, wv, bv, wp, bp, t1=64, t2=64):
    H, W = int(t1), int(t2)
    nc = _get_nc(1)
    in_maps = make_in_maps(x, gn_w, gn_b, wq, bq, wk, bk, wv, bv, wp, bp)
    res = run_bass_kernel_spmd(nc, in_maps, core_ids=list(range(NCORES)))
    return assemble_out(res.results, H, W)


# revision 17
# speedup vs baseline: 1.2317x; 1.2317x over previous
"""Trainium2 Bass kernel for an AttentionBlock (GroupNorm -> q/k/v 1x1 conv ->
full S x S attention -> proj 1x1 conv -> residual).

Problem shapes: x [4, 512, 64, 64] fp32, S = 4096 tokens, C = 512 channels,
GroupNorm with 32 groups of 16 channels.

Sharding: 8 cores = 4 batches x 2 query-halves. Core c handles batch c//2 and
query rows [half*2048, (half+1)*2048). Each core of a batch-pair redundantly
computes k/v for its batch (cheap vs attention) so no collectives are needed.

Math/precision design (validated against the reference in numpy, rel-maxerr
~9e-3 vs the 2e-2 gate):
  * GroupNorm folded into the q/k/v weights: h = scale_c * x + shift_c, so
    q = (wq*scale) @ x + (bq + wq @ shift), etc. Stats are computed from the
    fp8 copy of x (adds ~0.1% to variance - negligible).
  * k's bias adds a per-query softmax constant -> cancels, never computed.
  * v's bias times sum_j(p_j) = 1 -> folded into the proj bias.
  * All big matmuls run in fp8e4m3 with DoubleRow perf mode (2 fp8 values per
    PE cell, contraction 256/instruction at 0.5 cycles/row).
  * Softmax: ex' = exp(s/sqrt(C) - KOFF) written by the ACT engine DIRECTLY to
    fp8 (KOFF chosen so max ex' ~ 30 << 240 = e4m3 max, and every row-max
    stays in normal range). The denominator is the PE-summed (ones-matmul,
    DoubleRow) total of the QUANTIZED ex' -> dividing after attn@v removes
    first-order quantization bias. Normalization: hs = ph * (1/den) on DVE.
  * proj runs f32r with f32r hs; residual + proj bias added by one DVE
    scalar_tensor_tensor per output tile.

Layouts (partition dim first; "DR pair" = 2 half-tiles packed for DoubleRow):
  x8  [m=2][cpair=128, u=2, s=4096]   channel c = m*256 + u*128 + p
  k8  [mo=2][128, 2, j=4096]          out-channel pairs, scores lhsT
  q8  [mo=2][128, 2, i=2048]          scores rhs
  v8  [j=128, t=16, u=2, c=512]       j = (2t+u)*128 + p, attn@v lhsT
  ex8 [j=128, t=16, u=2, i=512]       attn@v rhs (per i-block of 512)
"""

import os

import numpy as np
import ml_dtypes

import concourse.bacc as bacc
import concourse.tile as tile
from concourse import mybir
from concourse.bass_utils import run_bass_kernel_spmd

F32 = mybir.dt.float32
F32R = mybir.dt.float32r
FP8 = mybir.dt.float8e4
BF16 = mybir.dt.bfloat16
AF = mybir.ActivationFunctionType
OP = mybir.AluOpType
AX = mybir.AxisListType
DR = mybir.MatmulPerfMode.DoubleRow

C = 512
S = 4096
B = 4
NCORES = 8
CT = 4          # channel tiles of 128
CP = 2          # channel pair-tiles of 256 (DoubleRow)
SBLK = 8        # s-blocks of 512
QBLK = 4        # q-blocks of 512 (half = 2048 columns)
IB = 4          # i-blocks of 512 for attention
IBW = 512
JT = 32         # j-tiles of 128
TP = 16         # j pair-tiles of 256
HALF = S // 2
EPS = 1e-5
GELEMS = 16 * S                      # elements per group (16 ch x 4096)
SCL = 1.0 / np.sqrt(np.float32(C))   # softmax scale
KOFF = 4.0                           # exp offset: ex' = exp(s*SCL - KOFF)
# Timing-bisection variants (correctness only guaranteed for "full"):
#   full | nop5 (P1-P4 only) | p5a (no attnV/proj) | p5b (no scores/exp)
VARIANT = os.environ.get("KVARIANT", "full")


def build_nc(reps=1):
    nc = bacc.Bacc("TRN2", target_bir_lowering=False, debug=False,
                   num_devices=NCORES)

    x8_d = nc.dram_tensor("x8", [CP, 128, 2, S], FP8, kind="ExternalInput").ap()
    xh_d = nc.dram_tensor("xh", [CT, 128, HALF], F32, kind="ExternalInput").ap()
    wqt_d = nc.dram_tensor("wqt", [CT, 128, C], F32, kind="ExternalInput").ap()
    wkt_d = nc.dram_tensor("wkt", [CT, 128, C], F32, kind="ExternalInput").ap()
    wvt_d = nc.dram_tensor("wvt", [CT, 128, C], F32, kind="ExternalInput").ap()
    wpt_d = nc.dram_tensor("wpt", [CT, 128, C], F32R, kind="ExternalInput").ap()
    bq_d = nc.dram_tensor("bq", [CT, 128, 1], F32, kind="ExternalInput").ap()
    bv_d = nc.dram_tensor("bv", [CT, 128, 1], F32, kind="ExternalInput").ap()
    bp_d = nc.dram_tensor("bp", [CT, 128, 1], F32, kind="ExternalInput").ap()
    gnw_d = nc.dram_tensor("gnw", [CT, 128, 1], F32, kind="ExternalInput").ap()
    gnb_d = nc.dram_tensor("gnb", [CT, 128, 1], F32, kind="ExternalInput").ap()
    g16_d = nc.dram_tensor("g16", [128, 8], F32, kind="ExternalInput").ap()
    b8_d = nc.dram_tensor("b8", [8, 128], F32, kind="ExternalInput").ap()
    on8_d = nc.dram_tensor("on8", [128, 2, 128], FP8, kind="ExternalInput").ap()
    out_d = nc.dram_tensor("out", [CT, 128, HALF], F32, kind="ExternalOutput").ap()

    with tile.TileContext(nc) as tc:
        with tc.tile_pool(name="const", bufs=1) as cpool, \
             tc.tile_pool(name="resident", bufs=1) as rpool:
            g16_t = cpool.tile([128, 8], F32, name="g16t")
            b8_t = cpool.tile([8, 128], F32, name="b8t")
            on8_t = cpool.tile([128, 2, 128], FP8, name="on8t")
            onbf_t = cpool.tile([128, 128], BF16, name="onbft")
            eps_t = cpool.tile([8, 1], F32, name="epst")
            koff_t = cpool.tile([128, 1], F32, name="kofft")
            nc.sync.dma_start(g16_t[:], g16_d[:])
            nc.sync.dma_start(b8_t[:], b8_d[:])
            nc.sync.dma_start(on8_t[:], on8_d[:])
            nc.vector.memset(eps_t[:], EPS)
            nc.vector.memset(onbf_t[:], 1.0)
            nc.vector.memset(koff_t[:], -KOFF)
            gnw_t, gnb_t = [], []
            for ci in range(CT):
                gw = cpool.tile([128, 1], F32, name=f"gnw{ci}")
                gb = cpool.tile([128, 1], F32, name=f"gnb{ci}")
                nc.sync.dma_start(gw[:], gnw_d[ci])
                nc.sync.dma_start(gb[:], gnb_d[ci])
                gnw_t.append(gw)
                gnb_t.append(gb)

            for rep in range(reps):
                emit_rep(nc, tc, rpool, rep,
                         x8_d, xh_d, wqt_d, wkt_d, wvt_d, wpt_d,
                         bq_d, bv_d, bp_d,
                         g16_t, b8_t, onbf_t, eps_t, koff_t, gnw_t, gnb_t,
                         out_d)
    nc.compile()
    return nc


def emit_rep(nc, tc, rpool, rep, x8_d, xh_d, wqt_d, wkt_d, wvt_d, wpt_d,
             bq_d, bv_d, bp_d, g16_t, b8_t, onbf_t, eps_t, koff_t,
             gnw_t, gnb_t, out_d):
    # ---- resident tensors (slots shared across reps via fixed tags) ----
    k8 = [rpool.tile([128, 2, S], BF16, name=f"k8{m}_{rep}", tag=f"k8{m}")
          for m in range(CP)]
    q8 = [rpool.tile([128, 2, HALF], BF16, name=f"q8{m}_{rep}", tag=f"q8{m}")
          for m in range(CP)]
    v8 = rpool.tile([128, TP, 2, C], BF16, name=f"v8_{rep}", tag="v8")
    wpt_s = [rpool.tile([128, C], F32R, name=f"wp{ci}_{rep}", tag=f"wp{ci}")
             for ci in range(CT)]
    for ci in range(CT):
        nc.sync.dma_start(wpt_s[ci][:], wpt_d[ci])

    with tc.tile_pool(name=f"x8_{rep}", bufs=1) as x8pool, \
         tc.tile_pool(name=f"stat_{rep}", bufs=1) as spool, \
         tc.tile_pool(name=f"pstat_{rep}", bufs=1, space="PSUM") as pstats:

        x8_s = [x8pool.tile([128, 2, S], FP8, name=f"x8s{m}_{rep}", tag=f"x8m{m}")
                for m in range(CP)]
        for m in range(CP):
            nc.sync.dma_start(x8_s[m][:], x8_d[m])

        # ================= P1: per-channel sum / sumsq from fp8 x ============
        sq2 = spool.tile([128, CT, 2], F32, name=f"sq2_{rep}", tag="sq2")
        sqscr = spool.tile([128, S], FP8, name=f"sqscr_{rep}", tag="sqscr")
        for ci in range(CT):
            m, u = ci // 2, ci % 2
            xv = x8_s[m][:, u, :]
            nc.vector.reduce_sum(out=sq2[:, ci, 0:1], in_=xv, axis=AX.X)
            nc.scalar.activation(out=sqscr[:], in_=xv, func=AF.Square,
                                 accum_out=sq2[:, ci, 1:2])

        # ================= P2: group stats -> per-channel scale/shift =========
        gpsum = pstats.tile([8, 8], F32, name=f"gps_{rep}", tag="g")
        for ci in range(CT):
            nc.tensor.matmul(gpsum[:, 2 * ci:2 * ci + 2], g16_t[:], sq2[:, ci, :],
                             start=True, stop=True)
        gp3 = gpsum[:].rearrange("p (c t) -> p c t", t=2)
        packbuf = spool.tile([8, CT, 2], F32, name=f"pack_{rep}", tag="pack")
        ex2 = spool.tile([8, CT], F32, name=f"ex2_{rep}", tag="ex2")
        gm2 = spool.tile([8, CT], F32, name=f"gm2_{rep}", tag="gm2")
        gvar = spool.tile([8, CT], F32, name=f"gvar_{rep}", tag="gvar")
        nc.scalar.mul(out=packbuf[:, :, 1], in_=gp3[:, :, 0], mul=1.0 / GELEMS)
        nc.scalar.mul(out=ex2[:], in_=gp3[:, :, 1], mul=1.0 / GELEMS)
        nc.vector.tensor_mul(gm2[:], packbuf[:, :, 1], packbuf[:, :, 1])
        nc.vector.tensor_sub(gvar[:], ex2[:], gm2[:])
        nc.scalar.activation(out=gvar[:], in_=gvar[:], func=AF.Sqrt,
                             bias=eps_t[:], scale=1.0)
        nc.vector.reciprocal(out=packbuf[:, :, 0], in_=gvar[:])
        scale_t, shift_t = [], []
        for ci in range(CT):
            bca = pstats.tile([128, 2], F32, name=f"bca{ci}_{rep}", tag="bca")
            nc.tensor.matmul(bca[:], b8_t[:], packbuf[:, ci, :], start=True, stop=True)
            sc = spool.tile([128, 1], F32, name=f"scale{ci}_{rep}", tag=f"scale{ci}")
            sh = spool.tile([128, 1], F32, name=f"shift{ci}_{rep}", tag=f"shift{ci}")
            tm = spool.tile([128, 1], F32, name=f"tmpm{ci}_{rep}", tag="tmpm")
            nc.vector.tensor_mul(sc[:], gnw_t[ci][:], bca[:, 0:1])
            nc.vector.tensor_mul(tm[:], bca[:, 1:2], sc[:])
            nc.vector.tensor_sub(sh[:], gnb_t[ci][:], tm[:])
            scale_t.append(sc)
            shift_t.append(sh)

        # ================= P3: fold GN into fp8 weights + bias folds ==========
        with tc.tile_pool(name=f"w8_{rep}", bufs=1) as w8pool:
            wq8 = [w8pool.tile([128, 2, C], FP8, name=f"wq8{m}_{rep}", tag=f"wq8{m}")
                   for m in range(CP)]
            wk8 = [w8pool.tile([128, 2, C], FP8, name=f"wk8{m}_{rep}", tag=f"wk8{m}")
                   for m in range(CP)]
            wv8 = [w8pool.tile([128, 2, C], FP8, name=f"wv8{m}_{rep}", tag=f"wv8{m}")
                   for m in range(CP)]
            with tc.tile_pool(name=f"wfold_{rep}", bufs=1) as wfold:
                wq_s, wk_s, wv_s = [], [], []
                for nm, src, lst in (("wq", wqt_d, wq_s), ("wk", wkt_d, wk_s),
                                     ("wv", wvt_d, wv_s)):
                    for ci in range(CT):
                        w = wfold.tile([128, C], F32, name=f"{nm}{ci}_{rep}",
                                       tag=f"{nm}{ci}")
                        nc.sync.dma_start(w[:], src[ci])
                        lst.append(w)
                # bias folds with RAW weights: b' = b + w^T @ shift
                bq_sb, bv_sb = [], []
                for w_s, b_dram, lst, nm in ((wq_s, bq_d, bq_sb, "bq"),
                                             (wv_s, bv_d, bv_sb, "bv")):
                    for co in range(CT):
                        pb = pstats.tile([128, 1], F32, name=f"pb{nm}{co}_{rep}",
                                         tag="pb")
                        for ci in range(CT):
                            nc.tensor.matmul(
                                pb[:], w_s[ci][:, co * 128:(co + 1) * 128],
                                shift_t[ci][:], start=(ci == 0), stop=(ci == CT - 1))
                        braw = spool.tile([128, 1], F32, name=f"{nm}r{co}_{rep}",
                                          tag="braw")
                        nc.sync.dma_start(braw[:], b_dram[co])
                        bt = spool.tile([128, 1], F32, name=f"{nm}f{co}_{rep}",
                                        tag=f"{nm}f{co}")
                        nc.vector.tensor_add(bt[:], pb[:], braw[:])
                        lst.append(bt)
                # bp' = bp + wp^T @ bv'
                bp_sb = []
                for co in range(CT):
                    pb = pstats.tile([128, 1], F32, name=f"pbbp{co}_{rep}", tag="pb")
                    for ci in range(CT):
                        nc.tensor.matmul(
                            pb[:], wpt_s[ci][:].bitcast(F32)[:, co * 128:(co + 1) * 128],
                            bv_sb[ci][:], start=(ci == 0), stop=(ci == CT - 1))
                    braw = spool.tile([128, 1], F32, name=f"bpr{co}_{rep}", tag="braw")
                    nc.sync.dma_start(braw[:], bp_d[co])
                    bt = rpool.tile([128, 1], F32, name=f"bpf{co}_{rep}",
                                    tag=f"bpf{co}")
                    nc.vector.tensor_add(bt[:], pb[:], braw[:])
                    bp_sb.append(bt)
                # folded fp8 weights: w8[m][:, u, :] = w_s[2m+u] * scale[2m+u]
                # (on ACT - Copy with per-partition scale - to keep DVE free)
                for ws, w8t in ((wq_s, wq8), (wk_s, wk8), (wv_s, wv8)):
                    for ci in range(CT):
                        nc.scalar.activation(
                            out=w8t[ci // 2][:, ci % 2, :], in_=ws[ci][:],
                            func=AF.Copy, scale=scale_t[ci][:])

            # ================= P4: q / k / vT projections (DoubleRow) =========
            # k and q first (P5 stage-A needs them); v last with DVE stores so
            # the v work drains while P5's first exp-bound phase runs.
            with tc.tile_pool(name=f"pd_{rep}", bufs=5, space="PSUM") as pd:
                for sb in range(SBLK):
                    ssl = slice(sb * 512, (sb + 1) * 512)
                    for co in range(CT):
                        pk = pd.tile([128, 512], F32, name=f"pk{sb}{co}_{rep}",
                                     tag="pd")
                        for m in range(CP):
                            nc.tensor.matmul(pk[:],
                                             wk8[m][:, :, co * 128:(co + 1) * 128],
                                             x8_s[m][:, :, ssl], start=(m == 0),
                                             stop=(m == CP - 1), perf_mode=DR)
                        if co % 2 == 0:
                            nc.scalar.activation(out=k8[co // 2][:, co % 2, ssl],
                                                 in_=pk[:], func=AF.Copy)
                        else:
                            nc.vector.tensor_copy(k8[co // 2][:, co % 2, ssl],
                                                  pk[:])
                    if sb < QBLK:
                        for co in range(CT):
                            pq = pd.tile([128, 512], F32, name=f"pq{sb}{co}_{rep}",
                                         tag="pd")
                            for m in range(CP):
                                nc.tensor.matmul(pq[:],
                                                 wq8[m][:, :, co * 128:(co + 1) * 128],
                                                 x8_s[m][:, :, ssl], start=(m == 0),
                                                 stop=(m == CP - 1), perf_mode=DR)
                            nc.vector.tensor_scalar(
                                out=q8[co // 2][:, co % 2, ssl], in0=pq[:],
                                scalar1=bq_sb[co][:], scalar2=None, op0=OP.add)
                for jt in range(JT):
                    pv = pd.tile([128, 512], F32, name=f"pv{jt}_{rep}", tag="pd")
                    for m in range(CP):
                        nc.tensor.matmul(pv[:],
                                         x8_s[m][:, :, jt * 128:(jt + 1) * 128],
                                         wv8[m][:], start=(m == 0),
                                         stop=(m == CP - 1), perf_mode=DR)
                    nc.vector.tensor_copy(v8[:, jt // 2, jt % 2, :], pv[:])

    # ================= P5: attention + proj + residual (pipelined) ===========
    with tc.tile_pool(name=f"ex8_{rep}", bufs=2) as ex8pool, \
         tc.tile_pool(name=f"hn_{rep}", bufs=2) as hnpool, \
         tc.tile_pool(name=f"eo_{rep}", bufs=4) as eopool, \
         tc.tile_pool(name=f"psc_{rep}", bufs=3, space="PSUM") as psc, \
         tc.tile_pool(name=f"pph_{rep}", bufs=4, space="PSUM") as pph, \
         tc.tile_pool(name=f"psm_{rep}", bufs=1, space="PSUM") as psm:

        ex8_t, rbc_t = [None] * IB, [None] * IB

        def emit_slot(ia, ib):
            """One pipeline slot: stage-A of i-block `ia` (scores -> exp ->
            den) interleaved at js/t granularity with stage-B of i-block `ib`
            (attnV -> normalize -> proj -> residual). The interleave keeps the
            PE busy on attnV matmuls whenever the scores stream stalls on the
            (slower) ACT exp consumer, and vice versa."""
            if ia is not None:
                isl_a = slice(ia * IBW, (ia + 1) * IBW)
                ex8a = ex8pool.tile([128, TP, 2, IBW], BF16, name=f"ex{ia}_{rep}",
                                    tag="ex8")
                pdn = psm.tile([128, IBW], F32, name=f"pdn{ia}_{rep}", tag="sm")
            if ib is not None:
                isl_b = slice(ib * IBW, (ib + 1) * IBW)
                ex8b, rbcb = ex8_t[ib], rbc_t[ib]
                xrs = []
                for co in range(CT):
                    xr = eopool.tile([128, IBW], F32, name=f"xr{ib}{co}_{rep}",
                                     tag="xr")
                    nc.sync.dma_start(xr[:], xh_d[co, :, isl_b])
                    xrs.append(xr)
                ph = [pph.tile([128, IBW], F32, name=f"ph{ib}{ci}_{rep}", tag="ph")
                      for ci in range(CT)]
                hs = hnpool.tile([128, CT, IBW], F32R, name=f"hs{ib}_{rep}",
                                 tag="hs")
            for s in range(JT):
                if ia is not None:
                    ps_ = psc.tile([128, IBW], F32, name=f"ps{ia}{s}_{rep}",
                                   tag="ps")
                    for m in range(CP):
                        for u in range(2):
                            nc.tensor.matmul(
                                ps_[:], k8[m][:, u, s * 128:(s + 1) * 128],
                                q8[m][:, u, isl_a], start=(m == 0 and u == 0),
                                stop=(m == CP - 1 and u == 1))
                    nc.scalar.activation(out=ex8a[:, s // 2, s % 2, :], in_=ps_[:],
                                         func=AF.Exp, scale=float(SCL),
                                         bias=koff_t[:])
                    t, u = s // 2, s % 2
                    nc.tensor.matmul(pdn[:], onbf_t[:], ex8a[:, t, u, :],
                                     start=(s == 0), stop=(s == JT - 1),
                                     skip_group_check=True)
                if ib is not None:
                    for g in (2 * s, 2 * s + 1):
                        ci, t = g // TP, g % TP
                        for u in range(2):
                            nc.tensor.matmul(
                                ph[ci][:], v8[:, t, u, ci * 128:(ci + 1) * 128],
                                ex8b[:, t, u, :], start=(t == 0 and u == 0),
                                stop=(t == TP - 1 and u == 1),
                                skip_group_check=True)
                        if t == TP - 1:
                            nc.vector.tensor_mul(hs[:, ci, :], ph[ci][:], rbcb[:])
            if ia is not None:
                rbc = hnpool.tile([128, IBW], F32, name=f"rbc{ia}_{rep}",
                                  tag="rbc")
                nc.vector.reciprocal(out=rbc[:], in_=pdn[:])
                ex8_t[ia], rbc_t[ia] = ex8a, rbc
            if ib is not None:
                for co in range(CT):
                    pp = psc.tile([128, IBW], F32, name=f"pp{ib}{co}_{rep}",
                                  tag="ps")
                    for ci in range(CT):
                        nc.tensor.matmul(pp[:],
                                         wpt_s[ci][:, co * 128:(co + 1) * 128],
                                         hs[:, ci, :], start=(ci == 0),
                                         stop=(ci == CT - 1))
                    ot = eopool.tile([128, IBW], F32, name=f"ot{ib}{co}_{rep}",
                                     tag="ot")
                    nc.vector.scalar_tensor_tensor(out=ot[:], in0=pp[:],
                                                   scalar=bp_sb[co][:],
                                                   in1=xrs[co][:],
                                                   op0=OP.add, op1=OP.add)
                    nc.sync.dma_start(out_d[co, :, isl_b], ot[:])

        if VARIANT == "nop5":
            return
        if VARIANT == "p5a":
            for ib in range(IB):
                emit_slot(ib, None)
        elif VARIANT == "p5b":
            ex8c = rpool.tile([128, TP, 2, IBW], BF16, name=f"ex8c_{rep}",
                              tag="ex8c")
            rbcc = rpool.tile([128, IBW], F32, name=f"rbcc_{rep}", tag="rbcc")
            if rep == 0:
                nc.vector.memset(ex8c[:], 0.01)
                nc.vector.memset(rbcc[:], 1.0)
            for ib in range(IB):
                ex8_t[ib], rbc_t[ib] = ex8c, rbcc
            for ib in range(IB):
                emit_slot(None, ib)
        else:
            for ib in range(IB + 1):
                emit_slot(ib if ib < IB else None, ib - 1 if ib >= 1 else None)


# ---------------------------------------------------------------------------
# Host side
# ---------------------------------------------------------------------------
_NC_CACHE = {}


def _get_nc(reps=1):
    if reps not in _NC_CACHE:
        _NC_CACHE[reps] = build_nc(reps)
    return _NC_CACHE[reps]


def make_in_maps(x, gn_w, gn_b, wq, bq, wk, bk, wv, bv, wp, bp):
    xf = np.ascontiguousarray(np.asarray(x, dtype=np.float32)).reshape(B, C, S)
    g16 = np.zeros((128, 8), np.float32)
    g16[np.arange(128), np.arange(128) // 16] = 1.0
    b8 = np.ascontiguousarray(g16.T)
    shared = {
        "wqt": np.ascontiguousarray(np.asarray(wq, np.float32).T).reshape(CT, 128, C),
        "wkt": np.ascontiguousarray(np.asarray(wk, np.float32).T).reshape(CT, 128, C),
        "wvt": np.ascontiguousarray(np.asarray(wv, np.float32).T).reshape(CT, 128, C),
        "wpt": np.ascontiguousarray(np.asarray(wp, np.float32).T).reshape(CT, 128, C),
        "bq": np.asarray(bq, np.float32).reshape(CT, 128, 1),
        "bv": np.asarray(bv, np.float32).reshape(CT, 128, 1),
        "bp": np.asarray(bp, np.float32).reshape(CT, 128, 1),
        "gnw": np.asarray(gn_w, np.float32).reshape(CT, 128, 1),
        "gnb": np.asarray(gn_b, np.float32).reshape(CT, 128, 1),
        "g16": g16,
        "b8": b8,
        "on8": np.ones((128, 2, 128), ml_dtypes.float8_e4m3),
    }
    in_maps = []
    for core in range(NCORES):
        b, half = core // 2, core % 2
        xb = xf[b]
        if half == 0:
            xp = xb
        else:
            xp = np.concatenate([xb[:, HALF:], xb[:, :HALF]], axis=1)
        xp = np.ascontiguousarray(xp)
        # x8[m][p, u, s] = xp[m*256 + u*128 + p, s]
        x8 = np.ascontiguousarray(
            xp.reshape(CP, 2, 128, S).transpose(0, 2, 1, 3)
        ).astype(ml_dtypes.float8_e4m3)
        xh = np.ascontiguousarray(xp[:, :HALF]).reshape(CT, 128, HALF)
        in_maps.append(dict(shared, x8=x8, xh=xh))
    return in_maps


def assemble_out(results, H=64, W=64):
    out = np.empty((B, C, S), np.float32)
    for core in range(NCORES):
        b, half = core // 2, core % 2
        out[b][:, half * HALF:(half + 1) * HALF] = \
            results[core]["out"].reshape(C, HALF)
    return out.reshape(B, C, H, W)


def kernel(x, gn_w, gn_b, wq, bq, wk, bk, wv, bv, wp, bp, t1=64, t2=64):
    H, W = int(t1), int(t2)
    nc = _get_nc(1)
    in_maps = make_in_maps(x, gn_w, gn_b, wq, bq, wk, bk, wv, bv, wp, bp)
    res = run_bass_kernel_spmd(nc, in_maps, core_ids=list(range(NCORES)))
    return assemble_out(res.results, H, W)
